# revision 6
# baseline (speedup 1.0000x reference)
"""Self-contained Trainium2 Bass kernel for the 2-layer GCN encoder.

kernel(**inputs) takes FULL inputs (features [100000,128] f32,
edge_index [2,1600000] int, edge_weight [1600000] f32, W1,b1,a1,W2,b2,a2)
and returns the FULL [100000,128] f32 output, running on 8 NeuronCores.

Strategy (dst-sharded message passing, replicated weights):
  - nodes sharded across 8 cores (SH each, padded to NT*128)
  - per layer: GEMM on own shard -> AllGather h-table (bf16) -> dst-sharded
    aggregation: dma_gather h rows per 128-edge block, build a [128,512]
    one-hot*norm S block with one fused is_equal/mult tensor_scalar, and
    accumulate out[f, dst] = msgs^T @ S into a per-supertile PSUM bank.
  - all tunnel traffic is compressed: features bf16, gather indices shipped
    [16, TB*8] int16 and replicated to 128 partitions on device, dst rows
    uint16, norms bf16, output returned bf16 (host converts to f32).
"""
import os
os.environ.setdefault("NEURON_RT_RESET_CORES", "1")

import sys
sys.path.insert(0, "/opt/trn_rl_repo")
import numpy as np
from concourse import bacc, mybir, library_config
from concourse.bass_utils import run_bass_kernel_spmd

F32 = mybir.dt.float32
BF16 = mybir.dt.bfloat16
I16 = mybir.dt.int16
U16 = mybir.dt.uint16
I32 = mybir.dt.int32
NPBF = mybir.dt.np(BF16)

C = 8            # cores
D = 128          # feature dim
NPASS = 4        # src-range passes (int16 gather indices)
SUPER = 4        # dst tiles per super-tile (one PSUM bank per super)
SW = SUPER * 128  # dst nodes per super-tile
MAXIDX = 1024    # max indices per dma_gather call (HW ring limit)
RING = 32        # S-tile ring slots
GRP = 8          # S-ring sync granularity (blocks)


def _schedule(N, src, dst, norm):
    """Group edges into an SPMD-uniform static schedule (512-wide supers)."""
    SH = N // C
    NT = (SH + 127) // 128          # dst tiles per core
    SHP = NT * 128
    NS = (NT + SUPER - 1) // SUPER  # super tiles
    TBL = SHP * C
    PR = TBL // NPASS               # pass rows
    assert PR % 128 == 0 and PR * NPASS == TBL and PR <= 32767

    core = dst // SH
    dloc = dst - core * SH
    sup = dloc // SW
    srel = (dloc % SW).astype(np.uint16)
    tbl = (src // SH) * SHP + (src % SH)
    pas = tbl // PR
    prel = (tbl % PR).astype(np.int16)

    wsup = np.minimum(SW, SHP - np.arange(NS) * SW)   # dst width of super s

    key = (core * NS + sup) * NPASS + pas
    cnt = np.bincount(key, minlength=C * NS * NPASS).reshape(C, NS, NPASS)
    B = np.maximum(1, -(-cnt.max(axis=0) // 128))     # [NS, NPASS]
    Bf = B.reshape(-1)
    bbase = np.concatenate([[0], np.cumsum(Bf)]).astype(np.int64)
    TB = int(bbase[-1])                               # blocks per layer
    maxblk = int(B.max())

    # per-edge slot: blockbase(s,p)*128 + rank within (core,s,p) group
    ekey = sup * NPASS + pas
    gkey_all = (core * (NS * NPASS) + ekey).astype(np.int32)
    order = np.argsort(gkey_all, kind="stable")
    sc, se = core[order], ekey[order]
    gkey = gkey_all[order]
    first = np.empty(len(gkey), dtype=bool)
    first[0] = True
    np.not_equal(gkey[1:], gkey[:-1], out=first[1:])
    starts = np.where(first, np.arange(len(gkey)), 0)
    starts = np.maximum.accumulate(starts)
    rank = np.arange(len(gkey)) - starts
    slot = bbase[se] * 128 + rank

    idx_f = np.zeros((C, TB * 128), dtype=np.int16)
    dr_f = np.zeros((C, TB * 128), dtype=np.uint16)
    nm_f = np.zeros((C, TB * 128), dtype=np.float32)
    idx_f[sc, slot] = prel[order]
    dr_f[sc, slot] = srel[order]
    nm_f[sc, slot] = norm[order]

    # gather sub-calls: per (s,p), chunks of <= MAXIDX/128 blocks
    calls = []   # (s, p, fb(layer-block), nbl, queue)
    for s in range(NS):
        for p in range(NPASS):
            nb = int(B[s, p])
            fb0 = int(bbase[s * NPASS + p])
            off = 0
            while off < nb:
                n = min(nb - off, MAXIDX // 128)
                calls.append((s, p, fb0 + off, n, len(calls) % 4))
                off += n

    # cumulative blocks through sp (for stage3 double-buffer reuse waits)
    cum_sp = {sp: int(bbase[sp + 1]) for sp in range(NS * NPASS)}

    blk_first = np.zeros(TB, dtype=bool)
    blk_last = np.zeros(TB, dtype=bool)
    for s in range(NS):
        blk_first[int(bbase[s * NPASS])] = True
        blk_last[int(bbase[s * NPASS + NPASS - 1] + B[s, NPASS - 1] - 1)] = True

    return dict(
        N=N, SH=SH, NT=NT, SHP=SHP, NS=NS, TBL=TBL, PR=PR, TB=TB,
        B=B, bbase=bbase, wsup=wsup, calls=calls, maxblk=maxblk,
        cum_sp=cum_sp, blk_first=blk_first, blk_last=blk_last,
    ), idx_f, dr_f, nm_f


def _wrap_idx(idx_f):
    """[C, TB*128] int16 -> [C, 16, TB*8] wrapped-16 layout (compact)."""
    Cn, L = idx_f.shape
    return idx_f.reshape(Cn, L // 16, 16).transpose(0, 2, 1).copy()


def build_program(meta):
    NT, SHP, NS, TBL, PR, TB = (
        meta["NT"], meta["SHP"], meta["NS"], meta["TBL"], meta["PR"], meta["TB"]
    )
    B, bbase, wsup, calls, maxblk = (
        meta["B"], meta["bbase"], meta["wsup"], meta["calls"], meta["maxblk"]
    )
    cum_sp = meta["cum_sp"]
    blk_first, blk_last = meta["blk_first"], meta["blk_last"]
    NSP = NS * NPASS
    NI16 = TB * 8
    NGC = (NT + SUPER - 1) // SUPER   # gemm psum-copy groups (= NS)

    def gcum(spk):  # cumulative blocks through global sp-call spk
        return (spk // NSP) * TB + cum_sp[spk % NSP]

    nc = bacc.Bacc("TRN2", debug=False, num_swdge_queues=4)
    featT = nc.declare_dram_parameter("featT", [128, SHP], BF16, isOutput=False)
    idxs = nc.declare_dram_parameter("idxs", [16, NI16], I16, isOutput=False)
    drel = nc.declare_dram_parameter("drel", [128, TB], U16, isOutput=False)
    nrm = nc.declare_dram_parameter("nrm", [128, TB], BF16, isOutput=False)
    wbf = nc.declare_dram_parameter("wbf", [128, 256], BF16, isOutput=False)
    bpk = nc.declare_dram_parameter("bpk", [128, 4], F32, isOutput=False)
    out = nc.declare_dram_parameter("out", [128, SHP], BF16, isOutput=True)

    h_bounce = [nc.dram_tensor(f"h{l}_bounce", [SHP, D], BF16) for l in (1, 2)]
    h_table = [
        nc.dram_tensor(f"h{l}_table", [TBL, D], BF16, addr_space="Shared")
        for l in (1, 2)
    ]

    from contextlib import ExitStack
    with ExitStack() as ctx:
        ent = ctx.enter_context
        xbuf = ent(nc.sbuf_tensor("xbuf", [128, SHP], BF16))
        stage = ent(nc.sbuf_tensor("stage", [128, SHP], BF16))
        obuf = ent(nc.sbuf_tensor("obuf", [128, SHP], BF16))
        idx_sb = ent(nc.sbuf_tensor("idx_sb", [128, NI16], I16))
        drelf = ent(nc.sbuf_tensor("drelf", [128, TB], F32))
        nrmf = ent(nc.sbuf_tensor("nrmf", [128, TB], F32))
        sring = ent(nc.sbuf_tensor("sring", [128, RING, SW], BF16))
        iotaf = ent(nc.sbuf_tensor("iotaf", [128, SW], F32))
        iotai = ent(nc.sbuf_tensor("iotai", [128, SW], I32))
        w_sb = ent(nc.sbuf_tensor("w_sb", [128, 256], BF16))
        bpk_sb = ent(nc.sbuf_tensor("bpk_sb", [128, 4], F32))
        tpos = ent(nc.sbuf_tensor("tpos", [128, SW], F32))
        tneg = ent(nc.sbuf_tensor("tneg", [128, SW], F32))
        ps_all = ent(nc.psum_tensor("ps_all", [128, 8, 512], F32))
        s_load = ent(nc.semaphore("s_load"))
        s_gat = [ent(nc.semaphore(f"s_ga{i}")) for i in range(8)]
        s_dve = ent(nc.semaphore("s_dve"))
        s_pe = ent(nc.semaphore("s_pe"))
        s_peg = ent(nc.semaphore("s_peg"))
        s_dveg = ent(nc.semaphore("s_dveg"))
        s_iot = ent(nc.semaphore("s_iot"))
        s_post = ent(nc.semaphore("s_post"))
        s_store = ent(nc.semaphore("s_store"))
        s_cc = ent(nc.semaphore("s_cc"))
        block = ent(nc.Block())

        # obuf front region doubles as landing pad for compact drel/nrm
        dview = obuf[:, 0:TB].bitcast(U16)
        nview = obuf[:, TB:2 * TB].bitcast(BF16)

        def agg_ps(s):
            return ps_all[:, s % 4, :]

        def gemm_ps(tt):
            return ps_all[:, 4 + (tt % 2), :]

        stage3 = stage[:, : 2 * maxblk * 128].rearrange(
            "p (b f) -> p b f", f=128
        )

        def msg_ap(layer, gl, sp):
            buf = (layer * NSP + sp) % 2
            loc = gl - int(bbase[sp])
            return stage3[:, buf * maxblk + loc, :]

        def post(vector, layer, s):
            vector.wait_ge(s_pe, TB * layer + cum_sp[s * NPASS + NPASS - 1])
            w = int(wsup[s])
            ps = agg_ps(s)[:, 0:w]
            bo = 0 if layer == 0 else 2
            b_sb = bpk_sb[:, bo:bo + 1]
            a_sb = bpk_sb[:, bo + 1:bo + 2]
            dst = xbuf if layer == 0 else obuf
            vector.tensor_scalar(
                tpos[:, 0:w], ps, b_sb, 0.0,
                op0=mybir.AluOpType.add, op1=mybir.AluOpType.max,
            )
            vector.tensor_scalar(
                tneg[:, 0:w], ps, b_sb, 0.0,
                op0=mybir.AluOpType.add, op1=mybir.AluOpType.min,
            )
            vector.tensor_scalar(
                tneg[:, 0:w], tneg[:, 0:w], a_sb, None,
                op0=mybir.AluOpType.mult,
            )
            vector.tensor_tensor(
                dst[:, s * SW: s * SW + w], tpos[:, 0:w], tneg[:, 0:w],
                op=mybir.AluOpType.add,
            ).then_inc(s_post, 1)

        @block.sync
        def _(sync):
            for ap_d, ap_s in (
                (xbuf[:], featT[:]), (idx_sb[0:16, :], idxs[:]),
                (dview, drel[:]), (nview, nrm[:]),
                (w_sb[:], wbf[:]), (bpk_sb[:], bpk[:]),
            ):
                sync.dma_start(out=ap_d, in_=ap_s).then_inc(s_load, 16)
            sync.wait_ge(s_load, 96)
            sync.dma_start(
                out=idx_sb[16:32, :], in_=idx_sb[0:16, :]
            ).then_inc(s_load, 16)
            sync.wait_ge(s_load, 112)
            sync.dma_start(
                out=idx_sb[32:64, :], in_=idx_sb[0:32, :]
            ).then_inc(s_load, 16)
            sync.wait_ge(s_load, 128)
            sync.dma_start(
                out=idx_sb[64:128, :], in_=idx_sb[0:64, :]
            ).then_inc(s_load, 16)
            sync.wait_ge(s_dveg, NGC)
            sync.dma_start(
                out=h_bounce[0].ap().rearrange("(t p) f -> p t f", p=128),
                in_=stage[:, : NT * 128].rearrange("p (t f) -> p t f", f=128),
            ).then_inc(s_store, 16)
            sync.wait_ge(s_dveg, 2 * NGC)
            sync.dma_start(
                out=h_bounce[1].ap().rearrange("(t p) f -> p t f", p=128),
                in_=stage[:, : NT * 128].rearrange("p (t f) -> p t f", f=128),
            ).then_inc(s_store, 16)
            sync.wait_ge(s_post, 2 * NS)
            sync.dma_start(out=out[:], in_=obuf[:]).then_inc(s_store, 16)
            sync.wait_ge(s_store, 48)

        @block.gpsimd
        def _(gpsimd):
            gpsimd.load_library(library_config.mlp)
            gpsimd.iota(
                iotai[:], pattern=[[1, SW]], base=0, channel_multiplier=0
            ).then_inc(s_iot, 1)
            for layer in range(2):
                gpsimd.wait_ge(s_store, 16 * (layer + 1))
                gpsimd.collective_compute(
                    "AllGather",
                    mybir.AluOpType.bypass,
                    replica_groups=[list(range(C))],
                    ins=[h_bounce[layer][:]],
                    outs=[h_table[layer][:]],
                ).then_inc(s_cc)
                gpsimd.wait_ge(s_cc, layer + 1)
                for k, (s, p, fb, nbl, q) in enumerate(calls):
                    sp = s * NPASS + p
                    spk = layer * NSP + sp
                    kk = layer * len(calls) + k
                    if spk >= 2 and fb == int(bbase[sp]):
                        gpsimd.wait_ge(s_pe, gcum(spk - 2))
                    buf = spk % 2
                    loc = fb - int(bbase[sp])
                    gpsimd.dma_gather(
                        stage3[:, buf * maxblk + loc: buf * maxblk + loc + nbl, :],
                        h_table[layer][p * PR: (p + 1) * PR, :],
                        idx_sb[:, fb * 8: (fb + nbl) * 8],
                        nbl * 128,
                        nbl * 128,
                        D,
                        queue_num=q,
                    ).then_inc(s_gat[kk % 8], 16)

        @block.vector
        def _(vector):
            vector.wait_ge(s_load, 96)
            vector.wait_ge(s_iot, 1)
            vector.tensor_copy(iotaf[:], iotai[:])
            vector.tensor_copy(drelf[:], dview)
            vector.tensor_copy(nrmf[:], nview)
            for tt in range(NGC):
                vector.wait_ge(s_peg, min(SUPER * (tt + 1), NT))
                gw = min(SUPER * 128, NT * 128 - tt * SUPER * 128)
                vector.tensor_copy(
                    stage[:, tt * 512: tt * 512 + gw], gemm_ps(tt)[:, 0:gw]
                ).then_inc(s_dveg, 1)
            for layer in range(2):
                g0 = TB * layer
                for s in range(NS):
                    for p in range(NPASS):
                        sp = s * NPASS + p
                        fb0 = int(bbase[sp])
                        end = fb0 + int(B[s, p])
                        gl = fb0
                        while gl < end:
                            w8 = min(end - gl, GRP - (gl % GRP))
                            g = g0 + gl
                            if g + w8 > RING:
                                vector.wait_ge(s_pe, g + w8 - RING)
                            r = g % RING
                            s8 = sring[:, r: r + w8, :]
                            vector.tensor_tensor(
                                s8,
                                iotaf[:, None, :].broadcast_to([128, w8, SW]),
                                drelf[:, gl: gl + w8, None].broadcast_to(
                                    [128, w8, SW]
                                ),
                                op=mybir.AluOpType.is_equal,
                            )
                            vector.tensor_tensor(
                                s8, s8,
                                nrmf[:, gl: gl + w8, None].broadcast_to(
                                    [128, w8, SW]
                                ),
                                op=mybir.AluOpType.mult,
                            ).then_inc(s_dve, w8)
                            gl += w8
                    if s >= 1:
                        post(vector, layer, s - 1)
                post(vector, layer, NS - 1)
                if layer == 0:
                    for tt in range(NGC):
                        vector.wait_ge(s_peg, NT + min(SUPER * (tt + 1), NT))
                        gw = min(SUPER * 128, NT * 128 - tt * SUPER * 128)
                        vector.tensor_copy(
                            stage[:, tt * 512: tt * 512 + gw],
                            gemm_ps(tt)[:, 0:gw],
                        ).then_inc(s_dveg, 1)

        @block.tensor
        def _(tensor):
            tensor.wait_ge(s_load, 144)
            for t in range(NT):
                tt = t // SUPER
                if tt >= 2:
                    tensor.wait_ge(s_dveg, tt - 1)
                tensor.matmul(
                    gemm_ps(tt)[:, (t % SUPER) * 128: (t % SUPER) * 128 + 128],
                    xbuf[:, t * 128: (t + 1) * 128],
                    w_sb[:, 0:128],
                    start=True, stop=True,
                    skip_group_check=True,
                ).then_inc(s_peg, 1)
            for layer in range(2):
                g0 = TB * layer
                for k, (s, p, fb, nbl, q) in enumerate(calls):
                    sp = s * NPASS + p
                    kk = layer * len(calls) + k
                    tensor.wait_ge(s_gat[kk % 8], 16 * (kk // 8 + 1))
                    for gl in range(fb, fb + nbl):
                        g = g0 + gl
                        if gl % GRP == 0:
                            tensor.wait_ge(s_dve, min(g + GRP, g0 + TB))
                        tensor.matmul(
                            agg_ps(s),
                            msg_ap(layer, gl, sp),
                            sring[:, g % RING, :],
                            start=bool(blk_first[gl]),
                            stop=bool(blk_last[gl]),
                            skip_group_check=True,
                        ).then_inc(s_pe, 1)
                if layer == 0:
                    for t in range(NT):
                        tt = t // SUPER
                        if t == 0:
                            tensor.wait_ge(s_post, NS)
                        if tt >= 2:
                            tensor.wait_ge(s_dveg, NGC + tt - 1)
                        tensor.matmul(
                            gemm_ps(tt)[:, (t % SUPER) * 128:
                                        (t % SUPER) * 128 + 128],
                            xbuf[:, t * 128: (t + 1) * 128],
                            w_sb[:, 128:256],
                            start=True, stop=True,
                            skip_group_check=True,
                        ).then_inc(s_peg, 1)

    nc.compile()
    return nc


def prepare(features, edge_index, edge_weight, W1, b1, a1, W2, b2, a2):
    N, Dd = features.shape
    assert Dd == D
    src = np.asarray(edge_index[0], dtype=np.int64)
    dst = np.asarray(edge_index[1], dtype=np.int64)
    w = np.asarray(edge_weight, dtype=np.float32)

    deg = (np.bincount(dst, weights=w.astype(np.float64), minlength=N) + 1.0)
    dis = (1.0 / np.sqrt(deg)).astype(np.float32)
    norm = dis[src] * w * dis[dst]
    allsrc = np.concatenate([src, np.arange(N, dtype=np.int64)])
    alldst = np.concatenate([dst, np.arange(N, dtype=np.int64)])
    allnorm = np.concatenate([norm, (dis * dis).astype(np.float32)])

    meta, idx_f, dr_f, nm_f = _schedule(N, allsrc, alldst, allnorm)
    SH, SHP, TB = meta["SH"], meta["SHP"], meta["TB"]

    idx_w = _wrap_idx(idx_f)
    dr_w = dr_f.reshape(C, TB, 128).transpose(0, 2, 1).copy()
    nm_w = nm_f.reshape(C, TB, 128).transpose(0, 2, 1).astype(NPBF)

    featT = np.zeros((C, 128, SHP), dtype=NPBF)
    fbf = np.asarray(features, dtype=np.float32).astype(NPBF)
    for c in range(C):
        featT[c, :, :SH] = fbf[c * SH:(c + 1) * SH].T

    wpack = np.concatenate(
        [np.asarray(W1, np.float32), np.asarray(W2, np.float32)], axis=1
    ).astype(NPBF)
    bpack = np.stack(
        [np.asarray(b1, np.float32), np.asarray(a1, np.float32),
         np.asarray(b2, np.float32), np.asarray(a2, np.float32)], axis=1
    ).astype(np.float32)

    in_maps = []
    for c in range(C):
        in_maps.append(dict(
            featT=featT[c], idxs=idx_w[c], drel=dr_w[c], nrm=nm_w[c],
            wbf=wpack, bpk=bpack,
        ))
    return meta, in_maps


def kernel(features, edge_index, edge_weight, W1, b1, a1, W2, b2, a2):
    meta, in_maps = prepare(
        features, edge_index, edge_weight, W1, b1, a1, W2, b2, a2
    )
    nc = build_program(meta)
    res = run_bass_kernel_spmd(nc, in_maps, core_ids=list(range(C))).results
    SH = meta["SH"]
    return np.concatenate(
        [r["out"].T[:SH].astype(np.float32) for r in res], axis=0
    )


# revision 12
# speedup vs baseline: 1.2294x; 1.2294x over previous
"""Self-contained Trainium2 Bass kernel for the 2-layer GCN encoder.

kernel(**inputs) takes FULL inputs (features [100000,128] f32,
edge_index [2,1600000] int, edge_weight [1600000] f32, W1,b1,a1,W2,b2,a2)
and returns the FULL [100000,128] f32 output, running on 8 NeuronCores.

Strategy (dst-sharded message passing, replicated weights):
  - nodes sharded across 8 cores (SH each, padded to NT*128)
  - per layer: GEMM on own shard -> AllGather h-table (bf16) -> dst-sharded
    aggregation: dma_gather h rows per 128-edge block, build a [128,512]
    one-hot*norm S block with one fused is_equal/mult tensor_scalar, and
    accumulate out[f, dst] = msgs^T @ S into a per-supertile PSUM bank.
  - all tunnel traffic is compressed: features bf16, gather indices shipped
    [16, TB*8] int16 and replicated to 128 partitions on device, dst rows
    uint16, norms bf16, output returned bf16 (host converts to f32).
"""
import os
os.environ.setdefault("NEURON_RT_RESET_CORES", "1")

import sys
sys.path.insert(0, "/opt/trn_rl_repo")
import numpy as np
from concourse import bacc, mybir, library_config
from concourse.bass_utils import run_bass_kernel_spmd

F32 = mybir.dt.float32
BF16 = mybir.dt.bfloat16
I16 = mybir.dt.int16
U16 = mybir.dt.uint16
I32 = mybir.dt.int32
NPBF = mybir.dt.np(BF16)

C = 8            # cores
D = 128          # feature dim
NPASS = 4        # src-range passes (int16 gather indices)
SUPER = 4        # dst tiles per super-tile (one PSUM bank per super)
SW = SUPER * 128  # dst nodes per super-tile
MAXIDX = 1024    # max indices per dma_gather call (HW ring limit)
RING = 32        # S-tile ring slots
GRP = 16         # S-ring sync granularity (blocks)


def _schedule(N, src, dst, norm):
    """Group edges into an SPMD-uniform static schedule (512-wide supers)."""
    SH = N // C
    NT = (SH + 127) // 128          # dst tiles per core
    SHP = NT * 128
    NS = (NT + SUPER - 1) // SUPER  # super tiles
    TBL = SHP * C
    PR = TBL // NPASS               # pass rows
    assert PR % 128 == 0 and PR * NPASS == TBL and PR <= 32767

    core = dst // SH
    dloc = dst - core * SH
    sup = dloc // SW
    srel = (dloc % SW).astype(np.uint16)
    tbl = (src // SH) * SHP + (src % SH)
    pas = tbl // PR
    prel = (tbl % PR).astype(np.int16)

    wsup = np.minimum(SW, SHP - np.arange(NS) * SW)   # dst width of super s

    key = (core * NS + sup) * NPASS + pas
    cnt = np.bincount(key, minlength=C * NS * NPASS).reshape(C, NS, NPASS)
    B = np.maximum(1, -(-cnt.max(axis=0) // 128))     # [NS, NPASS]
    B[NS - 1, NPASS - 1] += (-int(B.sum())) % GRP     # align TB to GRP
    Bf = B.reshape(-1)
    bbase = np.concatenate([[0], np.cumsum(Bf)]).astype(np.int64)
    TB = int(bbase[-1])                               # blocks per layer
    maxblk = int(B.max())

    # per-edge slot: blockbase(s,p)*128 + rank within (core,s,p) group
    ekey = sup * NPASS + pas
    gkey_all = (core * (NS * NPASS) + ekey).astype(np.int32)
    order = np.argsort(gkey_all, kind="stable")
    sc, se = core[order], ekey[order]
    gkey = gkey_all[order]
    first = np.empty(len(gkey), dtype=bool)
    first[0] = True
    np.not_equal(gkey[1:], gkey[:-1], out=first[1:])
    starts = np.where(first, np.arange(len(gkey)), 0)
    starts = np.maximum.accumulate(starts)
    rank = np.arange(len(gkey)) - starts
    slot = bbase[se] * 128 + rank

    idx_f = np.zeros((C, TB * 128), dtype=np.int16)
    dr_f = np.zeros((C, TB * 128), dtype=np.uint16)
    nm_f = np.zeros((C, TB * 128), dtype=np.float32)
    idx_f[sc, slot] = prel[order]
    dr_f[sc, slot] = srel[order]
    nm_f[sc, slot] = norm[order]

    # gather sub-calls: per (s,p), chunks of <= MAXIDX/128 blocks
    calls = []   # (s, p, fb(layer-block), nbl, queue)
    for s in range(NS):
        for p in range(NPASS):
            nb = int(B[s, p])
            fb0 = int(bbase[s * NPASS + p])
            off = 0
            while off < nb:
                n = min(nb - off, MAXIDX // 128)
                calls.append((s, p, fb0 + off, n, len(calls) % 4))
                off += n

    # cumulative blocks through sp (for stage3 double-buffer reuse waits)
    cum_sp = {sp: int(bbase[sp + 1]) for sp in range(NS * NPASS)}

    blk_first = np.zeros(TB, dtype=bool)
    blk_last = np.zeros(TB, dtype=bool)
    for s in range(NS):
        blk_first[int(bbase[s * NPASS])] = True
        blk_last[int(bbase[s * NPASS + NPASS - 1] + B[s, NPASS - 1] - 1)] = True

    return dict(
        N=N, SH=SH, NT=NT, SHP=SHP, NS=NS, TBL=TBL, PR=PR, TB=TB,
        B=B, bbase=bbase, wsup=wsup, calls=calls, maxblk=maxblk,
        cum_sp=cum_sp, blk_first=blk_first, blk_last=blk_last,
    ), idx_f, dr_f, nm_f


def _wrap_idx(idx_f):
    """[C, TB*128] int16 -> [C, 16, TB*8] wrapped-16 layout (compact)."""
    Cn, L = idx_f.shape
    return idx_f.reshape(Cn, L // 16, 16).transpose(0, 2, 1).copy()


def build_program(meta):
    NT, SHP, NS, TBL, PR, TB = (
        meta["NT"], meta["SHP"], meta["NS"], meta["TBL"], meta["PR"], meta["TB"]
    )
    B, bbase, wsup, calls, maxblk = (
        meta["B"], meta["bbase"], meta["wsup"], meta["calls"], meta["maxblk"]
    )
    cum_sp = meta["cum_sp"]
    blk_first, blk_last = meta["blk_first"], meta["blk_last"]
    NSP = NS * NPASS
    NI16 = TB * 8
    NGC = (NT + SUPER - 1) // SUPER   # gemm psum-copy groups (= NS)

    def gcum(spk):  # cumulative blocks through global sp-call spk
        return (spk // NSP) * TB + cum_sp[spk % NSP]

    nc = bacc.Bacc("TRN2", debug=False, num_swdge_queues=4)
    featT = nc.declare_dram_parameter("featT", [128, SHP], BF16, isOutput=False)
    idxs = nc.declare_dram_parameter("idxs", [16, NI16], I16, isOutput=False)
    edat = nc.declare_dram_parameter("edat", [128, 2 * TB], U16, isOutput=False)
    wbf = nc.declare_dram_parameter("wbf", [128, 256], BF16, isOutput=False)
    bpk = nc.declare_dram_parameter("bpk", [128, 4], F32, isOutput=False)
    out = nc.declare_dram_parameter("out", [128, SHP], BF16, isOutput=True)

    h_bounce = [nc.dram_tensor(f"h{l}_bounce", [SHP, D], BF16) for l in (1, 2)]
    h_table = [
        nc.dram_tensor(f"h{l}_table", [TBL, D], BF16, addr_space="Shared")
        for l in (1, 2)
    ]

    from contextlib import ExitStack
    with ExitStack() as ctx:
        ent = ctx.enter_context
        xbuf = ent(nc.sbuf_tensor("xbuf", [128, SHP], BF16))
        stage = ent(nc.sbuf_tensor("stage", [128, SHP], BF16))
        obuf = ent(nc.sbuf_tensor("obuf", [128, SHP], BF16))
        idx_sb = ent(nc.sbuf_tensor("idx_sb", [128, NI16], I16))
        drelf = ent(nc.sbuf_tensor("drelf", [128, TB], F32))
        nrmf = ent(nc.sbuf_tensor("nrmf", [128, TB], F32))
        sring = ent(nc.sbuf_tensor("sring", [128, RING, SW], BF16))
        iotaf = ent(nc.sbuf_tensor("iotaf", [128, SW], F32))
        iotai = ent(nc.sbuf_tensor("iotai", [128, SW], I32))
        w_sb = ent(nc.sbuf_tensor("w_sb", [128, 256], BF16))
        bpk_sb = ent(nc.sbuf_tensor("bpk_sb", [128, 4], F32))
        tpos = ent(nc.sbuf_tensor("tpos", [128, SW], F32))
        tneg = ent(nc.sbuf_tensor("tneg", [128, SW], F32))
        ps_all = ent(nc.psum_tensor("ps_all", [128, 8, 512], F32))
        s_load = ent(nc.semaphore("s_load"))
        s_gat = [ent(nc.semaphore(f"s_ga{i}")) for i in range(8)]
        s_dve = ent(nc.semaphore("s_dve"))
        s_pe = ent(nc.semaphore("s_pe"))
        s_peg = ent(nc.semaphore("s_peg"))
        s_dveg = ent(nc.semaphore("s_dveg"))
        s_iot = ent(nc.semaphore("s_iot"))
        s_post = ent(nc.semaphore("s_post"))
        s_store = ent(nc.semaphore("s_store"))
        s_cc = ent(nc.semaphore("s_cc"))
        block = ent(nc.Block())

        # obuf front region doubles as landing pad for compact drel/nrm
        dview = obuf[:, 0:TB].bitcast(U16)
        nview = obuf[:, TB:2 * TB].bitcast(BF16)

        def agg_ps(s):
            return ps_all[:, s % 4, :]

        def gemm_ps(tt):
            return ps_all[:, 4 + (tt % 2), :]

        stage3 = stage[:, : 2 * maxblk * 128].rearrange(
            "p (b f) -> p b f", f=128
        )

        def msg_ap(layer, gl, sp):
            buf = (layer * NSP + sp) % 2
            loc = gl - int(bbase[sp])
            return stage3[:, buf * maxblk + loc, :]

        def post(vector, layer, s):
            vector.wait_ge(s_pe, TB * layer + cum_sp[s * NPASS + NPASS - 1])
            w = int(wsup[s])
            ps = agg_ps(s)[:, 0:w]
            bo = 0 if layer == 0 else 2
            b_sb = bpk_sb[:, bo:bo + 1]
            a_sb = bpk_sb[:, bo + 1:bo + 2]
            dst = xbuf if layer == 0 else obuf
            vector.tensor_scalar(
                tpos[:, 0:w], ps, b_sb, 0.0,
                op0=mybir.AluOpType.add, op1=mybir.AluOpType.max,
            )
            vector.tensor_scalar(
                tneg[:, 0:w], ps, b_sb, 0.0,
                op0=mybir.AluOpType.add, op1=mybir.AluOpType.min,
            )
            vector.tensor_scalar(
                tneg[:, 0:w], tneg[:, 0:w], a_sb, None,
                op0=mybir.AluOpType.mult,
            )
            vector.tensor_tensor(
                dst[:, s * SW: s * SW + w], tpos[:, 0:w], tneg[:, 0:w],
                op=mybir.AluOpType.add,
            ).then_inc(s_post, 1)

        @block.sync
        def _(sync):
            for ap_d, ap_s in (
                (xbuf[:], featT[:]), (idx_sb[0:16, :], idxs[:]),
                (obuf[:, 0:2 * TB].bitcast(U16), edat[:]),
                (w_sb[:], wbf[:]), (bpk_sb[:], bpk[:]),
            ):
                sync.dma_start(out=ap_d, in_=ap_s).then_inc(s_load, 16)
            sync.wait_ge(s_load, 80)
            sync.dma_start(
                out=idx_sb[16:32, :], in_=idx_sb[0:16, :]
            ).then_inc(s_load, 16)
            sync.wait_ge(s_load, 96)
            sync.dma_start(
                out=idx_sb[32:64, :], in_=idx_sb[0:32, :]
            ).then_inc(s_load, 16)
            sync.wait_ge(s_load, 112)
            sync.dma_start(
                out=idx_sb[64:128, :], in_=idx_sb[0:64, :]
            ).then_inc(s_load, 16)
            sync.wait_ge(s_dveg, NGC)
            sync.dma_start(
                out=h_bounce[0].ap().rearrange("(t p) f -> p t f", p=128),
                in_=stage[:, : NT * 128].rearrange("p (t f) -> p t f", f=128),
            ).then_inc(s_store, 16)
            sync.wait_ge(s_dveg, 2 * NGC)
            sync.dma_start(
                out=h_bounce[1].ap().rearrange("(t p) f -> p t f", p=128),
                in_=stage[:, : NT * 128].rearrange("p (t f) -> p t f", f=128),
            ).then_inc(s_store, 16)
            sync.wait_ge(s_post, 2 * NS)
            sync.dma_start(out=out[:], in_=obuf[:]).then_inc(s_store, 16)
            sync.wait_ge(s_store, 48)

        @block.gpsimd
        def _(gpsimd):
            gpsimd.load_library(library_config.mlp)
            gpsimd.iota(
                iotai[:], pattern=[[1, SW]], base=0, channel_multiplier=0
            ).then_inc(s_iot, 1)
            for layer in range(2):
                gpsimd.wait_ge(s_store, 16 * (layer + 1))
                gpsimd.collective_compute(
                    "AllGather",
                    mybir.AluOpType.bypass,
                    replica_groups=[list(range(C))],
                    ins=[h_bounce[layer][:]],
                    outs=[h_table[layer][:]],
                ).then_inc(s_cc)
                gpsimd.wait_ge(s_cc, layer + 1)
                for k, (s, p, fb, nbl, q) in enumerate(calls):
                    sp = s * NPASS + p
                    spk = layer * NSP + sp
                    kk = layer * len(calls) + k
                    if spk >= 2 and fb == int(bbase[sp]):
                        gpsimd.wait_ge(s_pe, gcum(spk - 2))
                    buf = spk % 2
                    loc = fb - int(bbase[sp])
                    gpsimd.dma_gather(
                        stage3[:, buf * maxblk + loc: buf * maxblk + loc + nbl, :],
                        h_table[layer][p * PR: (p + 1) * PR, :],
                        idx_sb[:, fb * 8: (fb + nbl) * 8],
                        nbl * 128,
                        nbl * 128,
                        D,
                        queue_num=q,
                    ).then_inc(s_gat[kk % 8], 16)

        @block.vector
        def _(vector):
            vector.wait_ge(s_load, 80)
            vector.wait_ge(s_iot, 1)
            vector.tensor_copy(iotaf[:], iotai[:])
            vector.tensor_copy(drelf[:], dview)
            vector.tensor_copy(nrmf[:], nview)
            for tt in range(NGC):
                vector.wait_ge(s_peg, min(SUPER * (tt + 1), NT))
                gw = min(SUPER * 128, NT * 128 - tt * SUPER * 128)
                vector.tensor_copy(
                    stage[:, tt * 512: tt * 512 + gw], gemm_ps(tt)[:, 0:gw]
                ).then_inc(s_dveg, 1)
            for layer in range(2):
                g0 = TB * layer
                for s in range(NS):
                    for p in range(NPASS):
                        sp = s * NPASS + p
                        fb0 = int(bbase[sp])
                        end = fb0 + int(B[s, p])
                        gl = fb0
                        while gl < end:
                            w8 = min(end - gl, GRP - (gl % GRP))
                            g = g0 + gl
                            if g + w8 > RING:
                                vector.wait_ge(s_pe, g + w8 - RING)
                            r = g % RING
                            s8 = sring[:, r: r + w8, :]
                            vector.tensor_tensor(
                                s8,
                                iotaf[:, None, :].broadcast_to([128, w8, SW]),
                                drelf[:, gl: gl + w8, None].broadcast_to(
                                    [128, w8, SW]
                                ),
                                op=mybir.AluOpType.is_equal,
                            )
                            vector.tensor_tensor(
                                s8, s8,
                                nrmf[:, gl: gl + w8, None].broadcast_to(
                                    [128, w8, SW]
                                ),
                                op=mybir.AluOpType.mult,
                            ).then_inc(s_dve, w8)
                            gl += w8
                    if s >= 1:
                        post(vector, layer, s - 1)
                post(vector, layer, NS - 1)
                if layer == 0:
                    for tt in range(NGC):
                        vector.wait_ge(s_peg, NT + min(SUPER * (tt + 1), NT))
                        gw = min(SUPER * 128, NT * 128 - tt * SUPER * 128)
                        vector.tensor_copy(
                            stage[:, tt * 512: tt * 512 + gw],
                            gemm_ps(tt)[:, 0:gw],
                        ).then_inc(s_dveg, 1)

        @block.tensor
        def _(tensor):
            tensor.wait_ge(s_load, 128)
            for t in range(NT):
                tt = t // SUPER
                if tt >= 2:
                    tensor.wait_ge(s_dveg, tt - 1)
                tensor.matmul(
                    gemm_ps(tt)[:, (t % SUPER) * 128: (t % SUPER) * 128 + 128],
                    xbuf[:, t * 128: (t + 1) * 128],
                    w_sb[:, 0:128],
                    start=True, stop=True,
                    skip_group_check=True,
                ).then_inc(s_peg, 1)
            for layer in range(2):
                g0 = TB * layer
                for k, (s, p, fb, nbl, q) in enumerate(calls):
                    sp = s * NPASS + p
                    kk = layer * len(calls) + k
                    tensor.wait_ge(s_gat[kk % 8], 16 * (kk // 8 + 1))
                    for gl in range(fb, fb + nbl):
                        g = g0 + gl
                        if gl % GRP == 0:
                            tensor.wait_ge(s_dve, min(g + GRP, g0 + TB))
                        tensor.matmul(
                            agg_ps(s),
                            msg_ap(layer, gl, sp),
                            sring[:, g % RING, :],
                            start=bool(blk_first[gl]),
                            stop=bool(blk_last[gl]),
                            skip_group_check=True,
                        ).then_inc(s_pe, 1)
                if layer == 0:
                    for t in range(NT):
                        tt = t // SUPER
                        if t == 0:
                            tensor.wait_ge(s_post, NS)
                        if tt >= 2:
                            tensor.wait_ge(s_dveg, NGC + tt - 1)
                        tensor.matmul(
                            gemm_ps(tt)[:, (t % SUPER) * 128:
                                        (t % SUPER) * 128 + 128],
                            xbuf[:, t * 128: (t + 1) * 128],
                            w_sb[:, 128:256],
                            start=True, stop=True,
                            skip_group_check=True,
                        ).then_inc(s_peg, 1)

    nc.compile()
    return nc


def prepare(features, edge_index, edge_weight, W1, b1, a1, W2, b2, a2):
    N, Dd = features.shape
    assert Dd == D
    src = np.asarray(edge_index[0], dtype=np.int64)
    dst = np.asarray(edge_index[1], dtype=np.int64)
    w = np.asarray(edge_weight, dtype=np.float32)

    deg = (np.bincount(dst, weights=w.astype(np.float64), minlength=N) + 1.0)
    dis = (1.0 / np.sqrt(deg)).astype(np.float32)
    norm = dis[src] * w * dis[dst]
    allsrc = np.concatenate([src, np.arange(N, dtype=np.int64)])
    alldst = np.concatenate([dst, np.arange(N, dtype=np.int64)])
    allnorm = np.concatenate([norm, (dis * dis).astype(np.float32)])

    meta, idx_f, dr_f, nm_f = _schedule(N, allsrc, alldst, allnorm)
    SH, SHP, TB = meta["SH"], meta["SHP"], meta["TB"]

    idx_w = _wrap_idx(idx_f)
    dr_w = dr_f.reshape(C, TB, 128).transpose(0, 2, 1).copy()
    nm_w = nm_f.reshape(C, TB, 128).transpose(0, 2, 1).astype(NPBF)
    edat = np.concatenate([dr_w, nm_w.view(np.uint16)], axis=2)

    featT = np.zeros((C, 128, SHP), dtype=NPBF)
    fbf = np.asarray(features, dtype=np.float32).astype(NPBF)
    for c in range(C):
        featT[c, :, :SH] = fbf[c * SH:(c + 1) * SH].T

    wpack = np.concatenate(
        [np.asarray(W1, np.float32), np.asarray(W2, np.float32)], axis=1
    ).astype(NPBF)
    bpack = np.stack(
        [np.asarray(b1, np.float32), np.asarray(a1, np.float32),
         np.asarray(b2, np.float32), np.asarray(a2, np.float32)], axis=1
    ).astype(np.float32)

    in_maps = []
    for c in range(C):
        in_maps.append(dict(
            featT=featT[c], idxs=idx_w[c], edat=edat[c],
            wbf=wpack, bpk=bpack,
        ))
    return meta, in_maps


def kernel(features, edge_index, edge_weight, W1, b1, a1, W2, b2, a2):
    meta, in_maps = prepare(
        features, edge_index, edge_weight, W1, b1, a1, W2, b2, a2
    )
    nc = build_program(meta)
    res = run_bass_kernel_spmd(nc, in_maps, core_ids=list(range(C))).results
    SH = meta["SH"]
    return np.concatenate(
        [r["out"].T[:SH].astype(np.float32) for r in res], axis=0
    )
